# revision 1
# baseline (speedup 1.0000x reference)
"""Cross-attention (B=2, N=M=2048, DIM=1024, H=16) on 8 TRN2 NeuronCores.

Sharding: tensor-parallel over heads. Core i owns heads 2i,2i+1 (128 of the
1024 q/k/v dims). Each core computes its heads' attention over all tokens,
then an on-device AllToAll redistributes head-outputs so core i holds all
1024 dims for tokens [512i, 512(i+1)); each core then applies the full
output projection for its token slice. Host unshard is a pure concat.

Compute dtype: bf16 matmul operands, f32 PSUM accumulation.
"""

import sys

for _p in ("/opt/trn_rl_repo",):
    if _p not in sys.path:
        sys.path.append(_p)

import ml_dtypes
import numpy as np

import concourse.bass as bass
import concourse.mybir as mybir
import concourse.tile as tile
from concourse import bacc

NCORES = 8
B, N, M, DIM, H = 2, 2048, 2048, 1024, 16
D = DIM // H                  # 64 head dim
HPC = H // NCORES             # 2 heads per core
DLOC = HPC * D                # 128 local q/k/v dims per core
TOK = B * N                   # 4096 query tokens (flattened b-major)
MTOK = B * M                  # 4096 key tokens
TSL = TOK // NCORES           # 512-token output slice per core
SCALE = float(D) ** -0.5

KT = DIM // 128               # 8 contraction tiles for projections
NB = 512                      # matmul moving free dim / psum bank width
SW = 1024                     # scores psum width (2 banks)
MT = M // 128                 # 16 m-tiles per batch
NCH = TOK // NB               # 8 token chunks of 512 (all-to-all granularity)

BF16 = mybir.dt.bfloat16
F32 = mybir.dt.float32
AF = mybir.ActivationFunctionType


def build(dbg=False):
    nc = bacc.Bacc("TRN2", target_bir_lowering=False, debug=False,
                   num_devices=NCORES)

    # All big inputs are host-pre-tiled so every DMA source is contiguous:
    # x?t[nb] is one [128, KT, NB] block — a single 1 MB DMA per 512-token
    # block (DMA issue on the Sync engine costs ~0.7 us each, so few big
    # DMAs beat many small ones).
    x1t = nc.declare_dram_parameter("x1t", [TOK // NB, 128, KT, NB], BF16,
                                    isOutput=False)
    x2t = nc.declare_dram_parameter("x2t", [MTOK // NB, 128, KT, NB], BF16,
                                    isOutput=False)
    wq = nc.declare_dram_parameter("wq", [KT, 128, DLOC], BF16, isOutput=False)
    wk = nc.declare_dram_parameter("wk", [KT, 128, DLOC], BF16, isOutput=False)
    wv = nc.declare_dram_parameter("wv", [KT, 128, DLOC], BF16, isOutput=False)
    wp = nc.declare_dram_parameter("wp", [KT, 128, DIM], BF16, isOutput=False)
    bp = nc.declare_dram_parameter("bp", [1, DIM], BF16, isOutput=False)
    out = nc.declare_dram_parameter("out", [TSL, DIM], F32, isOutput=True)

    # DRAM bounce buffers for the AllToAll (collectives can't touch I/O).
    ata_in = nc.dram_tensor("ata_in", [NCORES, DLOC, TSL], BF16)
    ata_out = nc.dram_tensor("ata_out", [NCORES, DLOC, TSL], BF16)

    dbg_t = {}
    if dbg:
        dbg_t["qt"] = nc.declare_dram_parameter("dbg_qt", [128, TOK], BF16, isOutput=True)
        dbg_t["kt"] = nc.declare_dram_parameter("dbg_kt", [128, MTOK], BF16, isOutput=True)
        dbg_t["v"] = nc.declare_dram_parameter("dbg_v", [128, MTOK // 128, HPC, D + 1], BF16, isOutput=True)
        dbg_t["ot"] = nc.declare_dram_parameter("dbg_ot", [128, NCH, NB], BF16, isOutput=True)
        dbg_t["of"] = nc.declare_dram_parameter("dbg_of", [NCORES, DLOC, TSL], BF16, isOutput=True)
        dbg_t["o0"] = nc.declare_dram_parameter("dbg_o0", [D + 1, NB], F32, isOutput=True)
        dbg_t["bc0"] = nc.declare_dram_parameter("dbg_bc0", [D, NB], F32, isOutput=True)
        dbg_t["s0"] = nc.declare_dram_parameter("dbg_s0", [128, SW], F32, isOutput=True)

    with tile.TileContext(nc) as tc:
        with (
            tc.tile_pool(name="persist", bufs=1) as pp,
            tc.tile_pool(name="xin", bufs=12) as xp,
            tc.tile_pool(name="work", bufs=3) as wkp,
            tc.tile_pool(name="norm", bufs=4) as np_,
            tc.tile_pool(name="yout", bufs=3) as yp,
        ):
            # ---- persistent SBUF tensors ----
            wq_sb = pp.tile([128, KT, DLOC], BF16, tag="wq")
            wk_sb = pp.tile([128, KT, DLOC], BF16, tag="wk")
            wv_sb = pp.tile([128, KT, DLOC], BF16, tag="wv")
            wp_sb = pp.tile([128, KT, DIM], BF16, tag="wp")
            bp_sb = pp.tile([1, DIM], BF16, tag="bp")
            ones1 = pp.tile([1, 128], BF16, tag="ones1")
            # per-batch tensors so phase C(b) only depends on phase B(b)
            qt_b = [pp.tile([128, N], BF16, tag=f"qt{b}", name=f"qt{b}")
                    for b in range(B)]
            kt_b = [pp.tile([128, M], BF16, tag=f"kt{b}", name=f"kt{b}")
                    for b in range(B)]
            # v with ones column: [m-part, m-chunk, head, d+1]
            v_b = [pp.tile([128, M // 128, HPC, D + 1], BF16, tag=f"v{b}",
                           name=f"v{b}")
                   for b in range(B)]
            ot_sb = pp.tile([128, NCH, NB], BF16, tag="ot")  # normalized head out

            # wq on sync (needed first); the rest issue from other engines'
            # DGE queues so they don't delay the x1/x2 loads on sync
            for k in range(KT):
                nc.sync.dma_start(wq_sb[:, k, :], wq[k])
            for k in range(KT):
                nc.gpsimd.dma_start(wk_sb[:, k, :], wk[k])
                nc.gpsimd.dma_start(wv_sb[:, k, :], wv[k])
                nc.scalar.dma_start(wp_sb[:, k, :], wp[k])
            nc.gpsimd.dma_start(bp_sb[:], bp[:])
            nc.vector.memset(ones1[:], 1.0)
            for b in range(B):
                nc.vector.memset(v_b[b][:, :, :, D], 1.0)

            # ---- phases B (projections) and C (attention), interleaved
            # per batch: B0, C0, B1, C1 — B(b+1)'s DMAs and matmuls fill
            # C(b)'s ACT-bound slack. PSUM budget: B 2 banks (shared tag),
            # scores 4, attn-out 2, so B and C coexist.
            with (
                tc.tile_pool(name="ps_b", bufs=2, space="PSUM") as psb,
                tc.tile_pool(name="ps_s", bufs=2, space="PSUM") as pss,
                tc.tile_pool(name="ps_o", bufs=2, space="PSUM") as pso,
            ):

                def emit_b_chunk(b, i):
                    for nb in [(N // NB) * b + i]:
                        lsl = slice(NB * (nb % (N // NB)),
                                    NB * (nb % (N // NB)) + NB)
                        # qT [128 dloc, 512 tok] = sum_k wq_k.T @ x1t_k
                        x1_t = xp.tile([128, KT, NB], BF16, tag="x1", bufs=3,
                                       name="x1_t")
                        nc.sync.dma_start(x1_t[:], x1t[nb])
                        q_ps = psb.tile([128, NB], F32, tag="bps", name="q_ps")
                        for k in range(KT):
                            nc.tensor.matmul(q_ps[:], wq_sb[:, k, :],
                                             x1_t[:, k, :],
                                             start=(k == 0), stop=(k == KT - 1))
                        nc.vector.tensor_copy(qt_b[b][:, lsl], q_ps[:])

                        # kT same, from x2t; keep x2 tiles for v
                        x2_t = xp.tile([128, KT, NB], BF16, tag="x2", bufs=3,
                                       name="x2_t")
                        nc.sync.dma_start(x2_t[:], x2t[nb])
                        k_ps = psb.tile([128, NB], F32, tag="bps", name="k_ps")
                        for k in range(KT):
                            nc.tensor.matmul(k_ps[:], wk_sb[:, k, :],
                                             x2_t[:, k, :],
                                             start=(k == 0), stop=(k == KT - 1))
                        nc.vector.tensor_copy(kt_b[b][:, lsl], k_ps[:])

                        # v [m, dloc] per 128-chunk: lhsT = x2t chunk, rhs = wv
                        for j in range(NB // 128):
                            mc = (NB * nb) // 128 + j  # global m-chunk index
                            v_ps = psb.tile([128, NB], F32, tag="bps",
                                            name="v_ps")
                            for k in range(KT):
                                nc.tensor.matmul(
                                    v_ps[:, 0:DLOC],
                                    x2_t[:, k, 128 * j:128 * (j + 1)],
                                    wv_sb[:, k, :],
                                    start=(k == 0), stop=(k == KT - 1))
                            for hh in range(HPC):
                                nc.vector.tensor_copy(
                                    v_b[mc // (M // 128)][:, mc % (M // 128), hh, 0:D],
                                    v_ps[:, D * hh:D * (hh + 1)])

                def emit_c(b, filler=None):
                    for qb in range(N // NB):
                        if filler is not None:
                            filler(qb)
                        ch = (N * b) // NB + qb  # global 512-token chunk
                        o_ps = [pso.tile([D + 1, NB], F32, tag="ops",
                                         name="o_ps")
                                for _ in range(HPC)]
                        lnsl = slice(NB * qb, NB * (qb + 1))
                        pts = []
                        for mt in range(MT):
                            msl = slice(128 * mt, 128 * (mt + 1))
                            s_ps = pss.tile([128, HPC * NB], F32, tag="sps",
                                            name="s_ps")
                            for hh in range(HPC):
                                hsl = slice(D * hh, D * (hh + 1))
                                nc.tensor.matmul(
                                    s_ps[:, NB * hh:NB * (hh + 1)],
                                    kt_b[b][hsl, msl],
                                    qt_b[b][hsl, lnsl],
                                    start=True, stop=True)
                            pt = wkp.tile([128, HPC * NB], BF16, tag="pt",
                                          bufs=18, name="pt")
                            if dbg and b == 0 and qb == 0 and mt == 0:
                                s_stage = wkp.tile([128, HPC * NB], F32,
                                                   tag="s_stage")
                                nc.any.tensor_copy(s_stage[:], s_ps[:])
                                nc.sync.dma_start(dbg_t["s0"][:], s_stage[:])
                            nc.scalar.activation(pt[:], s_ps[:], AF.Exp,
                                                 scale=SCALE)
                            pts.append(pt)
                        # av matmuls emitted as one block AFTER all scores so
                        # a blocked av (waiting on o_ps slots) never sits in
                        # front of the next scores in the in-order PE stream
                        for mt in range(MT):
                            for hh in range(HPC):
                                nc.tensor.matmul(
                                    o_ps[hh][:],
                                    v_b[b][:, mt, hh, :],
                                    pts[mt][:, NB * hh:NB * (hh + 1)],
                                    start=(mt == 0), stop=(mt == MT - 1))
                        for hh in range(HPC):
                            hsl = slice(D * hh, D * (hh + 1))
                            rc = np_.tile([1, NB], F32, tag="recip",
                                          name="rc")
                            nc.vector.reciprocal(rc[:], o_ps[hh][D:D + 1, :])
                            bc = np_.tile([D, NB], F32, tag="bcast", name="bc")
                            nc.gpsimd.partition_broadcast(bc[:], rc[0:1, :])
                            if dbg and b == 0 and qb == 0 and hh == 0:
                                o_stage = wkp.tile([D + 1, NB], F32,
                                                   tag="o_stage")
                                nc.any.tensor_copy(o_stage[:], o_ps[hh][:])
                                nc.sync.dma_start(dbg_t["o0"][:], o_stage[:])
                                nc.sync.dma_start(dbg_t["bc0"][:], bc[:])
                            nc.vector.tensor_mul(
                                ot_sb[hsl, ch, :], o_ps[hh][0:D, :], bc[:])

                for i in range(N // NB):
                    emit_b_chunk(0, i)
                # B1's four 512-token chunks are interleaved at C0's
                # query-block boundaries, where the in-order PE stream has
                # slack under the ACT-bound exp pipeline
                emit_c(0, filler=lambda qb: emit_b_chunk(1, qb))
                emit_c(1)

            # ---- phase C': all-to-all over head-dim/token-chunks ----
            nc.sync.dma_start(
                ata_in[:].rearrange("c p t -> p c t"), ot_sb[:])
            nc.gpsimd.collective_compute(
                "AllToAll", mybir.AluOpType.bypass,
                replica_groups=[list(range(NCORES))],
                ins=[ata_in.ap().opt()],
                outs=[ata_out.ap().opt()],
            )
            of_tiles = []
            for k in range(NCORES):
                of = xp.tile([128, TSL], BF16, tag="of", bufs=8)
                nc.sync.dma_start(of[:], ata_out[k])
                of_tiles.append(of)

            if dbg:
                for b in range(B):
                    nc.sync.dma_start(dbg_t["qt"][:, N * b:N * (b + 1)], qt_b[b][:])
                    nc.sync.dma_start(dbg_t["kt"][:, M * b:M * (b + 1)], kt_b[b][:])
                    nc.sync.dma_start(
                        dbg_t["v"][:, (M // 128) * b:(M // 128) * (b + 1)], v_b[b][:])
                nc.sync.dma_start(dbg_t["ot"][:], ot_sb[:])
                for k in range(NCORES):
                    nc.sync.dma_start(dbg_t["of"][k], of_tiles[k][:])

            # ---- phase D: output projection for my 512-token slice ----
            with tc.tile_pool(name="ps_y", bufs=2, space="PSUM") as psy:
                # keep the PE busy (HAM-warm) while the collective flies;
                # results are never read
                dum_ps = psy.tile([128, NB], F32, tag="dum")
                for _ in range(56):
                    # reads the last ot chunk so these only start at C's end
                    nc.tensor.matmul(dum_ps[:], ot_sb[:, NCH - 1, 0:128],
                                     ot_sb[:, NCH - 1, :], start=True, stop=True)
                for tt in range(TSL // 128):
                    tsl_ = slice(128 * tt, 128 * (tt + 1))
                    for eb in range(DIM // NB):
                        esl = slice(NB * eb, NB * (eb + 1))
                        y_ps = psy.tile([128, NB], F32, tag="yps")
                        nc.tensor.matmul(y_ps[:], ones1[:], bp_sb[:, esl],
                                         start=True, stop=False)
                        for k in range(NCORES):
                            nc.tensor.matmul(y_ps[:], of_tiles[k][:, tsl_],
                                             wp_sb[:, k, esl],
                                             start=False, stop=(k == NCORES - 1))
                        y_sb = yp.tile([128, NB], F32, tag="ysb")
                        nc.vector.tensor_copy(y_sb[:], y_ps[:])
                        nc.sync.dma_start(out[tsl_, esl], y_sb[:])

    nc.compile()
    return nc


def _tile_xt(x):
    """[B,N,DIM] f32 -> [TOK//NB, 128, KT, NB] bf16 block-contiguous x^T."""
    bf = ml_dtypes.bfloat16
    xt = x.reshape(TOK, DIM).T  # [DIM, TOK]
    return np.ascontiguousarray(
        xt.reshape(KT, 128, TOK // NB, NB).transpose(2, 1, 0, 3)).astype(bf)


def make_in_maps(x1, x2, Wq, Wkv, Wproj, bproj):
    bf = ml_dtypes.bfloat16
    x1t = _tile_xt(x1)
    x2t = _tile_xt(x2)
    wk_full = Wkv[:, :DIM]
    wv_full = Wkv[:, DIM:]
    wp = np.ascontiguousarray(Wproj.reshape(KT, 128, DIM)).astype(bf)
    bp = bproj.reshape(1, DIM).astype(bf)
    in_maps = []
    for c in range(NCORES):
        sl = slice(DLOC * c, DLOC * (c + 1))
        in_maps.append({
            "x1t": x1t, "x2t": x2t,
            "wq": np.ascontiguousarray(Wq[:, sl]).reshape(KT, 128, DLOC).astype(bf),
            "wk": np.ascontiguousarray(wk_full[:, sl]).reshape(KT, 128, DLOC).astype(bf),
            "wv": np.ascontiguousarray(wv_full[:, sl]).reshape(KT, 128, DLOC).astype(bf),
            "wp": wp, "bp": bp,
        })
    return in_maps


_nc = None


def run(inputs, trace=False, dbg=False):
    """Returns (full_output [B,N,DIM] f32, BassKernelResults)."""
    global _nc
    from concourse.bass_utils import run_bass_kernel_spmd
    if _nc is None or dbg:
        _nc = build(dbg=dbg)
    in_maps = make_in_maps(**inputs)
    res = run_bass_kernel_spmd(_nc, in_maps, core_ids=list(range(NCORES)),
                               trace=trace)
    y = np.concatenate([res.results[c]["out"] for c in range(NCORES)], axis=0)
    return y.reshape(B, N, DIM), res


def kernel(x1, x2, Wq, Wkv, Wproj, bproj):
    y, _ = run(dict(x1=x1, x2=x2, Wq=Wq, Wkv=Wkv, Wproj=Wproj, bproj=bproj))
    return y



# revision 8
# speedup vs baseline: 1.0483x; 1.0483x over previous
"""Cross-attention (B=2, N=M=2048, DIM=1024, H=16) on 8 TRN2 NeuronCores.

Sharding: tensor-parallel over heads. Core i owns heads 2i,2i+1 (128 of the
1024 q/k/v dims). Attention is processed in two head-halves (A = head 2i,
B = head 2i+1): half A for all 8 token chunks first, then half B. This lets
the half-A AllToAll fly while half B computes, and the output projection is
split into a half-A part (woven into phase C_B) and a half-B part (tail).

Schedule: the PE stream is paced by the ACT engine's exp throughput
(~1.06us per [128,1024] tile). Projection matmuls ("weave quanta") are
emitted immediately before each dependent scores pair so the in-order PE
queue always has ready work while waiting for PSUM recycling.

Compute dtype: bf16 matmul operands, f32 PSUM accumulation.
"""

import sys

for _p in ("/opt/trn_rl_repo",):
    if _p not in sys.path:
        sys.path.append(_p)

from collections import deque

import ml_dtypes
import numpy as np

import concourse.bass as bass
import concourse.mybir as mybir
import concourse.tile as tile
from concourse import bacc

NCORES = 8
B, N, M, DIM, H = 2, 2048, 2048, 1024, 16
D = DIM // H                  # 64 head dim
HPC = H // NCORES             # 2 heads per core
DLOC = HPC * D                # 128 local q/k/v dims per core
TOK = B * N                   # 4096 query tokens (flattened b-major)
MTOK = B * M                  # 4096 key tokens
TSL = TOK // NCORES           # 512-token output slice per core
SCALE = float(D) ** -0.5

KT = DIM // 128               # 8 contraction tiles for projections
NB = 512                      # matmul moving free dim / psum bank width
MT = M // 128                 # 16 m-tiles per batch
NCH = TOK // NB               # 8 token chunks of 512

BF16 = mybir.dt.bfloat16
F32 = mybir.dt.float32
AF = mybir.ActivationFunctionType


def build():
    nc = bacc.Bacc("TRN2", target_bir_lowering=False, debug=False,
                   num_devices=NCORES)

    # Host-pre-tiled inputs: x?t[nb] is one [128, KT, NB] block -> a single
    # 1 MB contiguous DMA per 512-token block.
    x1t = nc.declare_dram_parameter("x1t", [TOK // NB, 128, KT, NB], BF16,
                                    isOutput=False)
    x2t = nc.declare_dram_parameter("x2t", [MTOK // NB, 128, KT, NB], BF16,
                                    isOutput=False)
    wq = nc.declare_dram_parameter("wq", [KT, 128, DLOC], BF16, isOutput=False)
    wk = nc.declare_dram_parameter("wk", [KT, 128, DLOC], BF16, isOutput=False)
    wv = nc.declare_dram_parameter("wv", [KT, 128, DLOC], BF16, isOutput=False)
    # wp_h[half]: Wproj rows for half-h head dims of all cores, regrouped so
    # contraction chunk k = [core 2k half-h dims | core 2k+1 half-h dims].
    wpa = nc.declare_dram_parameter("wpa", [KT // 2, 128, DIM], BF16,
                                    isOutput=False)
    wpb = nc.declare_dram_parameter("wpb", [KT // 2, 128, DIM], BF16,
                                    isOutput=False)
    bp = nc.declare_dram_parameter("bp", [1, DIM], F32, isOutput=False)
    out = nc.declare_dram_parameter("out", [TSL, DIM], F32, isOutput=True)

    # DRAM bounce buffers for the two half AllToAlls.
    ata_in = [nc.dram_tensor(f"ata_in{h}", [NCORES, D, TSL], BF16)
              for h in range(2)]
    ata_out = [nc.dram_tensor(f"ata_out{h}", [NCORES, D, TSL], BF16)
               for h in range(2)]

    with tile.TileContext(nc) as tc:
        with (
            tc.tile_pool(name="persist", bufs=1) as pp,
            tc.tile_pool(name="xin", bufs=12) as xp,
            tc.tile_pool(name="work", bufs=3) as wkp,
            tc.tile_pool(name="norm", bufs=4) as np_,
            tc.tile_pool(name="yout", bufs=2) as yp,
        ):
            # ---- persistent SBUF tensors ----
            wq_sb = pp.tile([128, KT, DLOC], BF16, tag="wq")
            wk_sb = pp.tile([128, KT, DLOC], BF16, tag="wk")
            wv_sb = pp.tile([128, KT, DLOC], BF16, tag="wv")
            wp_sb = [pp.tile([128, KT // 2, DIM], BF16, tag=f"wp{h}",
                             name=f"wp{h}")
                     for h in range(2)]
            bp_sb = pp.tile([1, DIM], F32, tag="bp")
            bias_bc = pp.tile([128, DIM], F32, tag="bias_bc")
            qt_b = [pp.tile([128, N], BF16, tag=f"qt{b}", name=f"qt{b}")
                    for b in range(B)]
            kt_b = [pp.tile([128, M], BF16, tag=f"kt{b}", name=f"kt{b}")
                    for b in range(B)]
            # v with ones column: [m-part, m-chunk, head, d+1]
            v_b = [pp.tile([128, M // 128, HPC, D + 1], BF16, tag=f"v{b}",
                           name=f"v{b}")
                   for b in range(B)]
            # rows 0:64 = half-A head outputs, 64:128 = half-B
            ot_sb = pp.tile([128, NCH, NB], BF16, tag="ot")
            # gathered head-outputs for my token slice, per half:
            # tile k rows = [core 2k | core 2k+1] half-h dims
            of_sb = [pp.tile([128, KT // 2, TSL], BF16, tag=f"of{h}",
                             name=f"of{h}")
                     for h in range(2)]
            # half-A projection partial sums (f32), [tok-part, tt, DIM]
            ya_sb = pp.tile([128, TSL // 128, DIM], F32, tag="ya")

            # weights on gpsimd DGE queue; x prefetches own sync
            for k in range(KT):
                nc.gpsimd.dma_start(wk_sb[:, k, :], wk[k])
            for k in range(KT):
                nc.gpsimd.dma_start(wq_sb[:, k, :], wq[k])
                nc.gpsimd.dma_start(wv_sb[:, k, :], wv[k])
            for k in range(KT // 2):
                nc.gpsimd.dma_start(wp_sb[0][:, k, :], wpa[k])
                nc.gpsimd.dma_start(wp_sb[1][:, k, :], wpb[k])
            nc.gpsimd.dma_start(bp_sb[:], bp[:])
            nc.gpsimd.partition_broadcast(bias_bc[:], bp_sb[0:1, :])
            for b in range(B):
                nc.vector.memset(v_b[b][:, :, :, D], 1.0)

            x_tiles = {}

            def fetch_x(which, b, nb):
                t = xp.tile([128, KT, NB], BF16, tag=f"x{which}", bufs=5,
                            name=f"x{which}_{b}{nb}")
                src = x1t if which == 1 else x2t
                nc.sync.dma_start(t[:], src[(N // NB) * b + nb])
                x_tiles[(which, b, nb)] = t

            with (
                tc.tile_pool(name="ps_s", bufs=2, space="PSUM") as pss,
                tc.tile_pool(name="ps_o", bufs=2, space="PSUM") as pso,
                tc.tile_pool(name="ps_b", bufs=2, space="PSUM") as psb,
            ):
                # ---------- weave quanta (emitted between scores) ----------
                def w_kproj(b, nb):
                    def emit():
                        x2_t = x_tiles[(2, b, nb)]
                        lsl = slice(NB * nb, NB * (nb + 1))
                        k_ps = psb.tile([128, NB], F32, tag="bps",
                                        name="k_ps")
                        for k in range(KT):
                            nc.tensor.matmul(k_ps[:], wk_sb[:, k, :],
                                             x2_t[:, k, :],
                                             start=(k == 0),
                                             stop=(k == KT - 1))
                        nc.vector.tensor_copy(kt_b[b][:, lsl], k_ps[:])
                    return emit

                def w_qproj(b, nb):
                    def emit():
                        x1_t = x_tiles[(1, b, nb)]
                        lsl = slice(NB * nb, NB * (nb + 1))
                        q_ps = psb.tile([128, NB], F32, tag="bps",
                                        name="q_ps")
                        for k in range(KT):
                            nc.tensor.matmul(q_ps[:], wq_sb[:, k, :],
                                             x1_t[:, k, :],
                                             start=(k == 0),
                                             stop=(k == KT - 1))
                        nc.vector.tensor_copy(qt_b[b][:, lsl], q_ps[:])
                    return emit

                def w_vproj(b, nb):
                    def emit():
                        x2_t = x_tiles[(2, b, nb)]
                        for j in range(NB // 128):
                            mc = nb * (NB // 128) + j  # m-chunk within batch
                            v_ps = psb.tile([128, NB], F32, tag="bps",
                                            name="v_ps")
                            for k in range(KT):
                                nc.tensor.matmul(
                                    v_ps[:, 0:DLOC],
                                    x2_t[:, k, 128 * j:128 * (j + 1)],
                                    wv_sb[:, k, :],
                                    start=(k == 0), stop=(k == KT - 1))
                            for hh in range(HPC):
                                nc.vector.tensor_copy(
                                    v_b[b][:, mc, hh, 0:D],
                                    v_ps[:, D * hh:D * (hh + 1)])
                    return emit

                def w_proj(half, tt, eb):
                    """Output projection for one (token-tile, e-block)."""
                    def emit():
                        tsl_ = slice(128 * tt, 128 * (tt + 1))
                        esl = slice(NB * eb, NB * (eb + 1))
                        y_ps = psb.tile([128, NB], F32, tag="bps",
                                        name="y_ps")
                        for k in range(KT // 2):
                            nc.tensor.matmul(y_ps[:],
                                             of_sb[half][:, k, tsl_],
                                             wp_sb[half][:, k, esl],
                                             start=(k == 0),
                                             stop=(k == KT // 2 - 1))
                        if half == 0:
                            # partial + bias -> f32 staging
                            nc.vector.tensor_add(ya_sb[:, tt, esl], y_ps[:],
                                                 bias_bc[:, esl])
                        else:
                            y_sb = yp.tile([128, NB], F32, tag="ysb")
                            nc.vector.tensor_add(y_sb[:], y_ps[:],
                                                 ya_sb[:, tt, esl])
                            nc.sync.dma_start(out[tsl_, esl], y_sb[:])
                    return emit

                weave = deque()

                def weave_one():
                    if weave:
                        weave.popleft()()

                # ---------- attention for one (half, b, qb) ----------
                def emit_attn(half, b, qb):
                    hsl = slice(D * half, D * (half + 1))
                    lnsl = slice(NB * qb, NB * (qb + 1))
                    ch = (N * b) // NB + qb
                    pts = []
                    for jp in range(MT // 2):      # mt pairs
                        weave_one()
                        s_ps = pss.tile([128, 2 * NB], F32, tag="sps",
                                        name="s_ps")
                        for j2 in range(2):
                            mt = 2 * jp + j2
                            msl = slice(128 * mt, 128 * (mt + 1))
                            nc.tensor.matmul(
                                s_ps[:, NB * j2:NB * (j2 + 1)],
                                kt_b[b][hsl, msl],
                                qt_b[b][hsl, lnsl],
                                start=True, stop=True)
                        pt = wkp.tile([128, 2 * NB], BF16, tag="pt",
                                      bufs=10, name="pt")
                        nc.scalar.activation(pt[:], s_ps[:], AF.Exp,
                                             scale=SCALE)
                        pts.append(pt)
                    o_ps = pso.tile([128, NB], F32, tag="ops", name="o_ps")
                    for jp in range(MT // 2):
                        for j2 in range(2):
                            mt = 2 * jp + j2
                            nc.tensor.matmul(
                                o_ps[0:D + 1, :],
                                v_b[b][:, mt, half, :],
                                pts[jp][:, NB * j2:NB * (j2 + 1)],
                                start=(mt == 0), stop=(mt == MT - 1))
                    rc = np_.tile([1, NB], F32, tag="recip", bufs=2,
                                  name="rc")
                    nc.vector.reciprocal(rc[:], o_ps[D:D + 1, :])
                    bc = np_.tile([D, NB], F32, tag="bcast", bufs=2,
                                  name="bc")
                    nc.gpsimd.partition_broadcast(bc[:], rc[0:1, :])
                    nc.vector.tensor_mul(ot_sb[hsl, ch, :],
                                         o_ps[0:D, :], bc[:])

                def emit_collective(half):
                    rsl = slice(D * half, D * (half + 1))
                    nc.sync.dma_start(
                        ata_in[half][:].rearrange("c p t -> p c t"),
                        ot_sb[rsl, :, :])
                    nc.gpsimd.collective_compute(
                        "AllToAll", mybir.AluOpType.bypass,
                        replica_groups=[list(range(NCORES))],
                        ins=[ata_in[half].ap().opt()],
                        outs=[ata_out[half].ap().opt()],
                    )
                    for k in range(KT // 2):
                        nc.sync.dma_start(of_sb[half][0:D, k, :],
                                          ata_out[half][2 * k])
                        nc.sync.dma_start(of_sb[half][D:128, k, :],
                                          ata_out[half][2 * k + 1])

                # ---------- emission schedule ----------
                # prologue: prefetch x2/x1 of b0, k-proj all b0, q-proj qb0
                for nb in range(4):
                    fetch_x(2, 0, nb)
                fetch_x(1, 0, 0)
                for nb in range(1, 4):
                    fetch_x(1, 0, nb)
                for nb in range(4):
                    fetch_x(2, 1, nb)
                for nb in range(4):
                    fetch_x(1, 1, nb)
                w_kproj(0, 0)()
                w_kproj(0, 1)()
                w_kproj(0, 2)()
                w_kproj(0, 3)()
                w_qproj(0, 0)()

                # C_A(b0): weave v_b0 (due by each AV), q_b0, then b1 proj
                weave.extend([
                    # during (0, qb0): v0 chunks due by AV of qb0
                    w_vproj(0, 0), w_vproj(0, 1), w_vproj(0, 2),
                    w_vproj(0, 3), w_qproj(0, 1),
                    # during (0, qb1..qb3): b1 k/q/v
                    w_qproj(0, 2), w_qproj(0, 3),
                    w_kproj(1, 0), w_kproj(1, 1), w_kproj(1, 2),
                    w_kproj(1, 3), w_qproj(1, 0),
                    w_vproj(1, 0), w_vproj(1, 1), w_vproj(1, 2),
                    w_vproj(1, 3), w_qproj(1, 1),
                    w_qproj(1, 2), w_qproj(1, 3),
                ])
                for b in range(B):
                    for qb in range(N // NB):
                        emit_attn(0, b, qb)
                emit_collective(0)

                # C_B: weave half-A projection (5 of 8 groups; 3 reserved
                # for the half-B collective window). The first ~2 qb worth
                # of slots stay empty so the proj matmuls don't reach the
                # head of the in-order PE queue before AllToAll#1 lands.
                noop = lambda: None  # noqa: E731
                weave.extend([noop] * 16)
                for tt in range(2):
                    for eb in range(2):
                        weave.append(w_proj(0, tt, eb))
                        weave.append(noop)
                weave.append(w_proj(0, 2, 0))
                for b in range(B):
                    for qb in range(N // NB):
                        emit_attn(1, b, qb)
                emit_collective(1)

                # keep PE warm with the reserved half-A proj groups while
                # the half-B collective flies
                w_proj(0, 2, 1)()
                w_proj(0, 3, 0)()
                w_proj(0, 3, 1)()
                while weave:
                    weave_one()

                # tail: half-B projection + output
                for tt in range(TSL // 128):
                    for eb in range(DIM // NB):
                        w_proj(1, tt, eb)()

    nc.compile()
    return nc


def _tile_xt(x):
    """[B,N,DIM] f32 -> [TOK//NB, 128, KT, NB] bf16 block-contiguous x^T."""
    bf = ml_dtypes.bfloat16
    xt = x.reshape(TOK, DIM).T  # [DIM, TOK]
    return np.ascontiguousarray(
        xt.reshape(KT, 128, TOK // NB, NB).transpose(2, 1, 0, 3)).astype(bf)


def make_in_maps(x1, x2, Wq, Wkv, Wproj, bproj):
    bf = ml_dtypes.bfloat16
    x1t = _tile_xt(x1)
    x2t = _tile_xt(x2)
    wk_full = Wkv[:, :DIM]
    wv_full = Wkv[:, DIM:]
    # wpa chunk k rows = [core 2k head-A dims | core 2k+1 head-A dims]
    wpr = Wproj.reshape(NCORES, 2, D, DIM)     # [core, half, d, e]
    wpa = np.ascontiguousarray(
        wpr[:, 0].reshape(KT // 2, 128, DIM)).astype(bf)
    wpb = np.ascontiguousarray(
        wpr[:, 1].reshape(KT // 2, 128, DIM)).astype(bf)
    bp = bproj.reshape(1, DIM).astype(np.float32)
    in_maps = []
    for c in range(NCORES):
        sl = slice(DLOC * c, DLOC * (c + 1))
        in_maps.append({
            "x1t": x1t, "x2t": x2t,
            "wq": np.ascontiguousarray(Wq[:, sl]).reshape(KT, 128, DLOC).astype(bf),
            "wk": np.ascontiguousarray(wk_full[:, sl]).reshape(KT, 128, DLOC).astype(bf),
            "wv": np.ascontiguousarray(wv_full[:, sl]).reshape(KT, 128, DLOC).astype(bf),
            "wpa": wpa, "wpb": wpb, "bp": bp,
        })
    return in_maps


_nc = None


def run(inputs, trace=False):
    """Returns (full_output [B,N,DIM] f32, BassKernelResults)."""
    global _nc
    from concourse.bass_utils import run_bass_kernel_spmd
    if _nc is None:
        _nc = build()
    in_maps = make_in_maps(**inputs)
    res = run_bass_kernel_spmd(_nc, in_maps, core_ids=list(range(NCORES)),
                               trace=trace)
    y = np.concatenate([res.results[c]["out"] for c in range(NCORES)], axis=0)
    return y.reshape(B, N, DIM), res


def kernel(x1, x2, Wq, Wkv, Wproj, bproj):
    y, _ = run(dict(x1=x1, x2=x2, Wq=Wq, Wkv=Wkv, Wproj=Wproj, bproj=bproj))
    return y


# revision 16
# speedup vs baseline: 1.0721x; 1.0227x over previous
"""Cross-attention (B=2, N=M=2048, DIM=1024, H=16) on 8 TRN2 NeuronCores.

Sharding: tensor-parallel over heads. Core i owns heads 2i,2i+1 (128 of the
1024 q/k/v dims). Attention runs in two head-halves (A = head 2i, B = head
2i+1): half A for all 8 token chunks, then half B, so the half-A AllToAll
flies while half B computes. Output projection = half-A part (woven into
phase C_B) + half-B part (tail).

Scheduling: PE p-state needs ~3us of gap-free execution to reach 2.4 GHz,
so the emission is software-pipelined at slot granularity: unit k's score
slots carry unit k-1's AV matmuls plus ~1 weave quantum (projection
matmuls) sized to slightly overfill the ACT exp cadence (~1.06us/tile).
The PE then never reaches a semaphore wait before it is satisfied.

Compute dtype: bf16 matmul operands, f32 PSUM accumulation.
"""

import sys

for _p in ("/opt/trn_rl_repo",):
    if _p not in sys.path:
        sys.path.append(_p)

from collections import deque

import ml_dtypes
import numpy as np

import concourse.mybir as mybir
import concourse.tile as tile
from concourse import bacc

NCORES = 8
B, N, M, DIM, H = 2, 2048, 2048, 1024, 16
D = DIM // H                  # 64 head dim
HPC = H // NCORES             # 2 heads per core
DLOC = HPC * D                # 128 local q/k/v dims per core
TOK = B * N                   # 4096 query tokens (flattened b-major)
MTOK = B * M                  # 4096 key tokens
TSL = TOK // NCORES           # 512-token output slice per core
SCALE = float(D) ** -0.5

KT = DIM // 128               # 8 contraction tiles for projections
NB = 512                      # matmul moving free dim / psum bank width
MT = M // 128                 # 16 m-tiles per batch
NCH = TOK // NB               # 8 token chunks of 512
NU = 16                       # units = (half, b, qb)
JP = MT // 2                  # 8 mt-pairs (slots) per unit

BF16 = mybir.dt.bfloat16
F32 = mybir.dt.float32
AF = mybir.ActivationFunctionType

# estimated PE cycles per slot @2.4GHz: scores pair (1024) + av pair (1024)
# vs ACT cadence ~1.06us = ~2550 cyc; overfill to ~2850.
SLOT_TARGET = 2850


def build():
    nc = bacc.Bacc("TRN2", target_bir_lowering=False, debug=False,
                   num_devices=NCORES)

    x1t = nc.declare_dram_parameter("x1t", [TOK // NB, 128, KT, NB], BF16,
                                    isOutput=False)
    x2t = nc.declare_dram_parameter("x2t", [MTOK // NB, 128, KT, NB], BF16,
                                    isOutput=False)
    wq = nc.declare_dram_parameter("wq", [KT, 128, DLOC], BF16, isOutput=False)
    wk = nc.declare_dram_parameter("wk", [KT, 128, DLOC], BF16, isOutput=False)
    wv = nc.declare_dram_parameter("wv", [KT, 128, DLOC], BF16, isOutput=False)
    wpa = nc.declare_dram_parameter("wpa", [KT // 2, 128, DIM], BF16,
                                    isOutput=False)
    wpb = nc.declare_dram_parameter("wpb", [KT // 2, 128, DIM], BF16,
                                    isOutput=False)
    bp = nc.declare_dram_parameter("bp", [1, DIM], F32, isOutput=False)
    out = nc.declare_dram_parameter("out", [TSL, DIM], F32, isOutput=True)

    ata_in = [nc.dram_tensor(f"ata_in{h}", [NCORES, D, TSL], BF16)
              for h in range(2)]
    ata_out = [nc.dram_tensor(f"ata_out{h}", [NCORES, D, TSL], BF16)
               for h in range(2)]

    with tile.TileContext(nc) as tc:
        with (
            tc.tile_pool(name="persist", bufs=1) as pp,
            tc.tile_pool(name="xin", bufs=8) as xp,
            tc.tile_pool(name="work", bufs=3) as wkp,
            tc.tile_pool(name="norm", bufs=2) as np_,
            tc.tile_pool(name="yout", bufs=2) as yp,
        ):
            wq_sb = pp.tile([128, KT, DLOC], BF16, tag="wq")
            wk_sb = pp.tile([128, KT, DLOC], BF16, tag="wk")
            wv_sb = pp.tile([128, KT, DLOC], BF16, tag="wv")
            wp_sb = [pp.tile([128, KT // 2, DIM], BF16, tag=f"wp{h}",
                             name=f"wp{h}")
                     for h in range(2)]
            bp_sb = pp.tile([1, DIM], F32, tag="bp")
            bias_bc = pp.tile([128, DIM], F32, tag="bias_bc")
            qt_b = [pp.tile([128, N], BF16, tag=f"qt{b}", name=f"qt{b}")
                    for b in range(B)]
            kt_b = [pp.tile([128, M], BF16, tag=f"kt{b}", name=f"kt{b}")
                    for b in range(B)]
            v_b = [pp.tile([128, M // 128, HPC, D + 1], BF16, tag=f"v{b}",
                           name=f"v{b}")
                   for b in range(B)]
            ot_sb = pp.tile([128, NCH, NB], BF16, tag="ot")
            of_sb = [pp.tile([128, KT // 2, TSL], BF16, tag=f"of{h}",
                             name=f"of{h}")
                     for h in range(2)]
            ya_sb = pp.tile([128, TSL // 128, DIM], F32, tag="ya")

            for k in range(2):
                nc.gpsimd.dma_start(wk_sb[:, k, :], wk[k])
            nc.gpsimd.dma_start(wq_sb[:, 0, :], wq[0])
            for k in range(2, KT):
                nc.gpsimd.dma_start(wk_sb[:, k, :], wk[k])
            for k in range(KT):
                if k > 0:
                    nc.gpsimd.dma_start(wq_sb[:, k, :], wq[k])
                nc.gpsimd.dma_start(wv_sb[:, k, :], wv[k])
            for k in range(KT // 2):
                nc.gpsimd.dma_start(wp_sb[0][:, k, :], wpa[k])
                nc.gpsimd.dma_start(wp_sb[1][:, k, :], wpb[k])
            nc.gpsimd.dma_start(bp_sb[:], bp[:])
            nc.gpsimd.partition_broadcast(bias_bc[:], bp_sb[0:1, :])
            for b in range(B):
                nc.vector.memset(v_b[b][:, :, :, D], 1.0)

            x_tiles = {}

            def fetch_x(which, b, nb):
                t = xp.tile([128, KT, NB], BF16, tag=f"x{which}", bufs=4,
                            name=f"x{which}_{b}{nb}")
                src = x1t if which == 1 else x2t
                nc.sync.dma_start(t[:], src[(N // NB) * b + nb])
                x_tiles[(which, b, nb)] = t

            with (
                tc.tile_pool(name="ps_s", bufs=2, space="PSUM") as pss,
                tc.tile_pool(name="ps_o", bufs=2, space="PSUM") as pso,
                tc.tile_pool(name="ps_b", bufs=2, space="PSUM") as psb,
            ):
                # ---------- weave quanta ----------
                # each group -> list of (est_cycles, fn); group psum tile is
                # created by the first quantum (shared via cell).

                def g_kq(kind, b, nb):
                    w_sb = wk_sb if kind == "k" else wq_sb
                    dst = kt_b[b] if kind == "k" else qt_b[b]
                    xw, xb = (2, b) if kind == "k" else (1, b)
                    cell = {}
                    quanta = []
                    for k0 in range(0, KT, 2):
                        def fn(k0=k0):
                            if k0 == 0:
                                cell["t"] = psb.tile([128, NB], F32,
                                                     tag="bps", name="kq_ps")
                            t = cell["t"]
                            xt = x_tiles[(xw, xb, nb)]
                            for k in (k0, k0 + 1):
                                nc.tensor.matmul(t[:], w_sb[:, k, :],
                                                 xt[:, k, :],
                                                 start=(k == 0),
                                                 stop=(k == KT - 1))
                            if k0 + 2 == KT:
                                nc.vector.tensor_copy(
                                    dst[:, NB * nb:NB * (nb + 1)], t[:])
                        quanta.append((1024, fn))
                    return quanta

                def g_v(b, nb):
                    quanta = []
                    for j in range(NB // 128):
                        def fn(j=j):
                            xt = x_tiles[(2, b, nb)]
                            mc = nb * (NB // 128) + j
                            v_ps = psb.tile([128, NB], F32, tag="bps",
                                            name="v_ps")
                            for k in range(KT):
                                nc.tensor.matmul(
                                    v_ps[:, 0:DLOC],
                                    xt[:, k, 128 * j:128 * (j + 1)],
                                    wv_sb[:, k, :],
                                    start=(k == 0), stop=(k == KT - 1))
                            for hh in range(HPC):
                                nc.vector.tensor_copy(
                                    v_b[b][:, mc, hh, 0:D],
                                    v_ps[:, D * hh:D * (hh + 1)])
                        quanta.append((1100, fn))
                    return quanta

                def g_proj(half, tt, eb):
                    cell = {}
                    quanta = []
                    for k0 in range(0, KT // 2, 2):
                        def fn(k0=k0):
                            tsl_ = slice(128 * tt, 128 * (tt + 1))
                            esl = slice(NB * eb, NB * (eb + 1))
                            if k0 == 0:
                                cell["t"] = psb.tile([128, NB], F32,
                                                     tag="bps", name="y_ps")
                            t = cell["t"]
                            for k in (k0, k0 + 1):
                                nc.tensor.matmul(t[:],
                                                 of_sb[half][:, k, tsl_],
                                                 wp_sb[half][:, k, esl],
                                                 start=(k == 0),
                                                 stop=(k == KT // 2 - 1))
                            if k0 + 2 == KT // 2:
                                if half == 0:
                                    nc.vector.tensor_add(
                                        ya_sb[:, tt, esl], t[:],
                                        bias_bc[:, esl])
                                else:
                                    y_sb = yp.tile([128, NB], F32,
                                                   tag="ysb")
                                    nc.vector.tensor_add(
                                        y_sb[:], t[:], ya_sb[:, tt, esl])
                                    nc.sync.dma_start(out[tsl_, esl],
                                                      y_sb[:])
                        quanta.append((1024, fn))
                    return quanta

                # ---------- due-ordered weave schedule ----------
                # units: 0..7 = half A (b0 q0..3, b1 q0..3), 8..15 = half B.
                # due = (unit, slot) BEFORE which the quantum must be done.
                # Emission order is semantic order, so the schedule is
                # stable-sorted by due before use.
                sched_items = []

                def add(due, quanta):
                    for q in quanta:
                        sched_items.append((due, len(sched_items), q))

                def add_fetch(due, which, b, nb):
                    add(due, [(0, lambda: fetch_x(which, b, nb))])

                # prologue (emitted directly): k(0,0) k(0,1) q(0,0)
                # k(b,nb) due (first unit of b, slot 2nb);
                # q(b,qb) due (unit of (b,qb), 0);
                # v(b,nb) due (unit after first unit of b, slot 2nb)
                add((0, 4), g_kq("k", 0, 2))
                add((0, 6), g_kq("k", 0, 3))
                add((1, 0), g_kq("q", 0, 1))
                add((1, 0), g_v(0, 0))
                add((1, 2), g_v(0, 1))
                add((1, 4), g_v(0, 2))
                add((1, 6), g_v(0, 3))
                add((2, 0), g_kq("q", 0, 2))
                # b1 x tiles fetched just-in-time: the fetch recycles a b0
                # x buffer, so it must be emitted after that buffer's last
                # reader (the b0 k/q/v quanta above).
                add_fetch((2, 0), 2, 1, 0)
                add_fetch((2, 4), 2, 1, 1)
                add((3, 0), g_kq("q", 0, 3))
                add_fetch((3, 0), 2, 1, 2)
                add_fetch((3, 2), 1, 1, 0)
                add_fetch((3, 4), 2, 1, 3)
                add_fetch((3, 6), 1, 1, 1)
                add((4, 0), g_kq("k", 1, 0))
                add((4, 0), g_kq("q", 1, 0))
                add((4, 2), g_kq("k", 1, 1))
                add_fetch((4, 2), 1, 1, 2)
                add((4, 4), g_kq("k", 1, 2))
                add((4, 6), g_kq("k", 1, 3))
                add_fetch((4, 6), 1, 1, 3)
                add((5, 0), g_v(1, 0))
                add((5, 0), g_kq("q", 1, 1))
                add((5, 2), g_v(1, 1))
                add((5, 4), g_v(1, 2))
                add((5, 6), g_v(1, 3))
                add((6, 0), g_kq("q", 1, 2))
                add((7, 0), g_kq("q", 1, 3))
                sched = deque(x for x in sorted(sched_items))
                # half-A projection: available once AllToAll#1 landed
                # (~unit 10); 6 groups woven, 2 reserved for the A2A#2
                # window. due never (=end) but min-unit enforced below.
                proj_a = deque()
                for tt in range(TSL // 128):
                    for eb in range(DIM // NB):
                        proj_a.append(g_proj(0, tt, eb))

                woven_proj = [0]

                def weave(u, jp, slot_cyc):
                    # force everything due before this slot
                    while sched and sched[0][0] <= (u, jp):
                        _, _, (cyc, fn) = sched.popleft()
                        fn()
                        slot_cyc += cyc
                    # fill to target; keep 2 proj groups (4 quanta) in
                    # reserve for the AllToAll#2 window
                    while slot_cyc < SLOT_TARGET:
                        if sched:
                            _, _, (cyc, fn) = sched.popleft()
                        elif u >= 12 and proj_a and woven_proj[0] < 12:
                            cyc, fn = proj_a[0].pop(0)
                            if not proj_a[0]:
                                proj_a.popleft()
                            woven_proj[0] += 1
                        else:
                            break
                        fn()
                        slot_cyc += cyc

                units = [(h, b, qb) for h in range(2) for b in range(B)
                         for qb in range(N // NB)]

                pts_prev = None
                unit_prev = None
                o_prev = None

                def emit_av_pair(uprev, pts, jp):
                    half, b, qb = uprev
                    for j2 in range(2):
                        mt = 2 * jp + j2
                        nc.tensor.matmul(
                            o_prev[0:D + 1, :],
                            v_b[b][:, mt, half, :],
                            pts[jp][:, NB * j2:NB * (j2 + 1)],
                            start=(mt == 0), stop=(mt == MT - 1))

                def emit_normalize(uprev):
                    half, b, qb = uprev
                    ch = (N * b) // NB + qb
                    hsl = slice(D * half, D * (half + 1))
                    rc = np_.tile([1, NB], F32, tag="recip", name="rc")
                    nc.vector.reciprocal(rc[:], o_prev[D:D + 1, :])
                    bc = np_.tile([D, NB], F32, tag="bcast", name="bc")
                    nc.gpsimd.partition_broadcast(bc[:], rc[0:1, :])
                    nc.vector.tensor_mul(ot_sb[hsl, ch, :],
                                         o_prev[0:D, :], bc[:])

                def emit_collective(half):
                    rsl = slice(D * half, D * (half + 1))
                    nc.sync.dma_start(
                        ata_in[half][:].rearrange("c p t -> p c t"),
                        ot_sb[rsl, :, :])
                    nc.gpsimd.collective_compute(
                        "AllToAll", mybir.AluOpType.bypass,
                        replica_groups=[list(range(NCORES))],
                        ins=[ata_in[half].ap().opt()],
                        outs=[ata_out[half].ap().opt()],
                    )
                    for k in range(KT // 2):
                        nc.sync.dma_start(of_sb[half][0:D, k, :],
                                          ata_out[half][2 * k])
                        nc.sync.dma_start(of_sb[half][D:128, k, :],
                                          ata_out[half][2 * k + 1])

                # ---------- prologue (b0 x tiles only) ----------
                for nb in range(4):
                    fetch_x(2, 0, nb)
                for nb in range(4):
                    fetch_x(1, 0, nb)
                for _, fn in g_kq("k", 0, 0) + g_kq("k", 0, 1) + \
                        g_kq("q", 0, 0):
                    fn()

                # ---------- pipelined units ----------
                for ui, unit in enumerate(units):
                    half, b, qb = unit
                    hsl = slice(D * half, D * (half + 1))
                    lnsl = slice(NB * qb, NB * (qb + 1))
                    o_cur = pso.tile([128, NB], F32, tag="ops", name="o_ps")
                    pts = []
                    for jp in range(JP):
                        weave(ui, jp, 2048 if pts_prev is not None else 1024)
                        s_ps = pss.tile([128, 2 * NB], F32, tag="sps",
                                        name="s_ps")
                        for j2 in range(2):
                            mt = 2 * jp + j2
                            msl = slice(128 * mt, 128 * (mt + 1))
                            nc.tensor.matmul(
                                s_ps[:, NB * j2:NB * (j2 + 1)],
                                kt_b[b][hsl, msl],
                                qt_b[b][hsl, lnsl],
                                start=True, stop=True)
                        pt = wkp.tile([128, 2 * NB], BF16, tag="pt",
                                      bufs=18, name="pt")
                        nc.scalar.activation(pt[:], s_ps[:], AF.Exp,
                                             scale=SCALE)
                        pts.append(pt)
                        if pts_prev is not None:
                            emit_av_pair(unit_prev, pts_prev, jp)
                    if pts_prev is not None:
                        emit_normalize(unit_prev)
                    pts_prev, unit_prev, o_prev = pts, unit, o_cur
                    if ui == 8:
                        # all half-A chunks normalized (unit 7's normalize
                        # was emitted during unit 8)
                        emit_collective(0)

                # drain: AV + normalize of the last unit
                for jp in range(JP):
                    emit_av_pair(unit_prev, pts_prev, jp)
                emit_normalize(unit_prev)
                emit_collective(1)

                # keep PE warm while AllToAll#2 flies
                while proj_a:
                    for cyc, fn in proj_a.popleft():
                        fn()
                while sched:
                    _, _, (cyc, fn) = sched.popleft()
                    fn()

                # tail: half-B projection + output
                for tt in range(TSL // 128):
                    for eb in range(DIM // NB):
                        for cyc, fn in g_proj(1, tt, eb):
                            fn()

    nc.compile()
    return nc


def _tile_xt(x):
    """[B,N,DIM] f32 -> [TOK//NB, 128, KT, NB] bf16 block-contiguous x^T."""
    bf = ml_dtypes.bfloat16
    xt = x.reshape(TOK, DIM).T
    return np.ascontiguousarray(
        xt.reshape(KT, 128, TOK // NB, NB).transpose(2, 1, 0, 3)).astype(bf)


def make_in_maps(x1, x2, Wq, Wkv, Wproj, bproj):
    bf = ml_dtypes.bfloat16
    x1t = _tile_xt(x1)
    x2t = _tile_xt(x2)
    wk_full = Wkv[:, :DIM]
    wv_full = Wkv[:, DIM:]
    wpr = Wproj.reshape(NCORES, 2, D, DIM)
    wpa = np.ascontiguousarray(
        wpr[:, 0].reshape(KT // 2, 128, DIM)).astype(bf)
    wpb = np.ascontiguousarray(
        wpr[:, 1].reshape(KT // 2, 128, DIM)).astype(bf)
    bp = bproj.reshape(1, DIM).astype(np.float32)
    in_maps = []
    for c in range(NCORES):
        sl = slice(DLOC * c, DLOC * (c + 1))
        in_maps.append({
            "x1t": x1t, "x2t": x2t,
            "wq": np.ascontiguousarray(Wq[:, sl]).reshape(KT, 128, DLOC).astype(bf),
            "wk": np.ascontiguousarray(wk_full[:, sl]).reshape(KT, 128, DLOC).astype(bf),
            "wv": np.ascontiguousarray(wv_full[:, sl]).reshape(KT, 128, DLOC).astype(bf),
            "wpa": wpa, "wpb": wpb, "bp": bp,
        })
    return in_maps


_nc = None


def run(inputs, trace=False):
    global _nc
    from concourse.bass_utils import run_bass_kernel_spmd
    if _nc is None:
        _nc = build()
    in_maps = make_in_maps(**inputs)
    res = run_bass_kernel_spmd(_nc, in_maps, core_ids=list(range(NCORES)),
                               trace=trace)
    y = np.concatenate([res.results[c]["out"] for c in range(NCORES)], axis=0)
    return y.reshape(B, N, DIM), res


def kernel(x1, x2, Wq, Wkv, Wproj, bproj):
    y, _ = run(dict(x1=x1, x2=x2, Wq=Wq, Wkv=Wkv, Wproj=Wproj, bproj=bproj))
    return y
